# Initial kernel scaffold
#
"""MicroTransformer (B=16,S=512,V=8000,D=5,F=20,L=2) on 8 trn2 NeuronCores.

Sharding: pure data parallel over batch (2 batch elements per core).
All parameters replicated. Whole transformer body + logits matmul run on
device; host only does input prep (embedding row gather, positional
encoding constant, weight layout transforms) and the final reshape.

Per-core device program (Bass/Tile, fully unrolled):
  state h [6, 1024] f32r: rows 0-4 = h^T for batch0|batch1, row 5 = ones
  (bias row for augmented matmuls).  All small constants arrive in one
  packed [21,174] DMA load; aug-tile ones rows / V zero-padding are
  DMA-loaded init patterns (single-row engine writes are illegal:
  engine APs must start at 32-aligned partitions).
  Attention is computed fully transposed ([k, q] layout), softmax without
  row-max (scores are bounded, |s| < 64 by construction of the inputs):
    qk       = Wqk_aug [6,37] x h-half  (q cols 0-4 pre-scaled by
               1/sqrt(D), k at cols 32-36 so its psum read is 32-aligned)
    scoresT  = k-chunk [5,128] x q [5,512]       -> PSUM [128k, 512q]
    eraw     = ACT Exp(scoresT)                  (PSUM -> SBUF f32)
    expT     = eraw * binary-causal-mask         (DVE, rounds to f32r)
    ctxZ     = sum_kc V_aug-chunk [128,33] x expT -> PSUM [33,512]
               (V has a ones column at 32 => partition 32 accumulates Z)
    1/Z      = reciprocal_approx_fast; bcast to [5,512] via a K=1 matmul
    proj/ffn = augmented f32r matmuls; LayerNorm via matmul stats
               (rsqrt = ACT Sqrt + DVE reciprocal_approx_fast)
  logits: final h is scattered to hfin4 [102,128] (4 seq-chunks at
  partition offsets 0/32/64/96); fcw4 holds fc_w_aug replicated at the
  same offsets.  Per 500-wide vocab chunk, 4 row-tiled matmuls
  (tile_position=(32i,0)) run concurrently in the PE array, each into
  its own single-bank psum tile (several row-tiled matmuls into one
  multi-bank tile crash at runtime).  Copies (2 DVE / 2 ACT) cast to
  fp16 stage tiles; per batch and 2-vocab-chunk group one contiguous
  1MB store (alternating HWDGE rings) writes out
  [128, 2, 8, 4000] fp16, so batch-0 stores overlap batch-1 tail work.
"""

import math

import numpy as np

import concourse.bacc as bacc
import concourse.bass as bass
import concourse.mybir as mybir
import concourse.tile as tile
from concourse.bass_utils import run_bass_kernel_spmd

F32 = mybir.dt.float32
F32R = mybir.dt.float32r
BF16 = mybir.dt.bfloat16
F16 = mybir.dt.float16
ALU = mybir.AluOpType
ACTF = mybir.ActivationFunctionType

def _r(ap):
    """float32r view: 4x PE throughput vs fp32 at moving size >= 256."""
    return ap.bitcast(F32R)


def _f(ap):
    """plain-f32 view (for tiny matmuls where fp32r is ISA-restricted)."""
    return ap.bitcast(F32)


B, S, V, D, F, L = 16, 512, 8000, 5, 20, 2
EPS = 1e-5
NCORES = 8
BPC = B // NCORES  # batches per core = 2
SQRT_D = math.sqrt(float(D))
SCALE = 1.0 / SQRT_D
QC = S // 128                  # 4 seq chunks of 128
VCH = 500                      # vocab chunk per matmul (<=512 psum bank)
NVC = V // VCH                 # 16
S2 = BPC * S                   # 1024

_CACHED = {}  # iters -> nc


def _build_program(iters=1):
    nc = bacc.Bacc("TRN2", target_bir_lowering=False, debug=False,
                   num_devices=NCORES)

    # ---- DRAM I/O ----
    d_h0 = nc.dram_tensor("h0", [D + 1, S2], F32R, kind="ExternalInput")
    d_pack = nc.dram_tensor("packw", [F + 1, 174], F32R,
                            kind="ExternalInput")
    d_mask = nc.dram_tensor("mask", [128, QC, S], F32, kind="ExternalInput")
    d_ctxa0 = nc.dram_tensor("ctxa0", [D + 1, S2], F32R,
                             kind="ExternalInput")
    d_f1a0 = nc.dram_tensor("f1a0", [F + 1, S2], F32R, kind="ExternalInput")
    d_vsb0 = nc.dram_tensor("vsb0", [128, QC, 33], F32R,
                            kind="ExternalInput")
    d_hf0 = nc.dram_tensor("hf0", [102, 128], F32R, kind="ExternalInput")
    d_fcw = nc.dram_tensor("fcw", [D + 1, V], F32R, kind="ExternalInput")
    d_out = nc.dram_tensor("out", [128, BPC, NVC // 2, 2 * QC * VCH],
                           F16, kind="ExternalOutput")

    from contextlib import ExitStack
    with tile.TileContext(nc) as tc, ExitStack() as es, \
            nc.allow_low_precision(reason="f32r/bf16/f16 rounding intended"):
        cst = es.enter_context(tc.tile_pool(name="cst", bufs=1))
        wrk = es.enter_context(tc.tile_pool(name="wrk", bufs=2))
        att = es.enter_context(tc.tile_pool(name="att", bufs=3))
        stg = es.enter_context(tc.tile_pool(name="stg", bufs=4))
        ps_big = es.enter_context(tc.tile_pool(name="ps_big", bufs=5,
                                               space="PSUM"))
        ps_sm = es.enter_context(tc.tile_pool(name="ps_sm", bufs=3,
                                              space="PSUM"))

        # ---- constants into SBUF (once) ----
        # ordering: h + one packed weight load go first on the sync ring so
        # the body can start immediately; the big mask/fcw4 loads follow.
        h = cst.tile([D + 1, S2], F32R, name="h", tag="h")
        nc.sync.dma_start(h[:], d_h0[:])
        pack = cst.tile([F + 1, 174], F32R, name="pack", tag="pack")
        nc.sync.dma_start(pack[:], d_pack[:])
        wqk = [pack[0:D + 1, 37 * l:37 * (l + 1)] for l in range(L)]
        wv = [_f(pack[0:D + 1, 74 + D * l:74 + D * (l + 1)])
              for l in range(L)]
        wo = [pack[0:D + 1, 84 + D * l:84 + D * (l + 1)] for l in range(L)]
        w1 = [pack[0:D + 1, 94 + F * l:94 + F * (l + 1)] for l in range(L)]
        w2 = [pack[0:F + 1, 134 + D * l:134 + D * (l + 1)] for l in range(L)]
        c02 = pack[0:D, 164:165]
        ones5 = pack[0:1, 169:169 + D]
        eps1 = cst.tile([1, 1], F32, name="eps1", tag="eps1")
        nc.vector.memset(eps1[:], EPS)
        maskb = cst.tile([128, QC, S], F32, name="maskb", tag="maskb")
        nc.scalar.dma_start(maskb[:], d_mask[:])
        fcw4 = cst.tile([102, V], F32R, name="fcw4", tag="fcw4")
        for i in range(QC):
            nc.sync.dma_start(fcw4[32 * i:32 * i + D + 1, :], d_fcw[:])
        # dedicated aug tiles (f32r): ones rows / zero padding loaded from
        # DRAM once (engine writes to single rows would be partition-
        # unaligned and memset can't write f32r); data rows are fully
        # rewritten every use, so the init patterns persist.
        ctxa = cst.tile([D + 1, S2], F32R, name="ctxa", tag="ctxa")
        nc.scalar.dma_start(ctxa[:], d_ctxa0[:])
        f1a = cst.tile([F + 1, S2], F32R, name="f1a", tag="f1a")
        nc.scalar.dma_start(f1a[:], d_f1a0[:])
        vsb = []
        for b in range(BPC):
            # V augmented: cols 0-4 = V, cols 5-31 = 0, col 32 = ones
            # => ctxZ matmul puts Z at out partition 32 (32-aligned read)
            t = cst.tile([128, QC, 33], F32R, name=f"vsb{b}", tag=f"vsb{b}")
            nc.scalar.dma_start(t[:], d_vsb0[:])
            vsb.append(t)
        hfin4 = []
        for b in range(BPC):
            t = cst.tile([102, 128], F32R, name=f"hfin4{b}", tag=f"hfin4{b}")
            nc.scalar.dma_start(t[:], d_hf0[:])
            hfin4.append(t)

        def layernorm(l, i, b, it, resid_ap, add_ps, out4=None, out_ap=None):
            """out = LN(resid + add) * g + b.

            out4: write result into hfin4[b] (4 chunk slices); else out_ap.
            """
            u = f"{l}{i}{b}_{it}"
            # chain kept on DVE where possible (same-engine deps avoid
            # cross-engine semaphore hops); only sqrt needs ACT
            x = wrk.tile([D, S], F32R, name=f"lnx{u}", tag="lnx")
            nc.vector.tensor_add(x[:], resid_ap, add_ps)
            xsq = wrk.tile([D, S], F32R, name=f"lnq{u}", tag="lnq")
            nc.vector.tensor_mul(xsq[:], x[:], x[:])
            s1 = ps_sm.tile([1, S], F32, name=f"s1_{u}", tag="ps_sm")
            s2 = ps_sm.tile([1, S], F32, name=f"s2_{u}", tag="ps_sm")
            nc.tensor.matmul(s1[:], c02, x[:])      # mean
            nc.tensor.matmul(s2[:], c02, xsq[:])    # E[x^2]
            t1 = wrk.tile([1, S], F32, name=f"t1_{u}", tag="lnt1")
            nc.scalar.square(t1[:], s1[:])             # mean^2 (ACT;
            # tensor_mul(s1, s1) would read PSUM twice in one op)
            var = wrk.tile([1, S], F32, name=f"lv_{u}", tag="lnvar")
            nc.vector.tensor_sub(var[:], s2[:], t1[:])
            se = wrk.tile([1, S], F32, name=f"se_{u}", tag="lnse")
            nc.scalar.activation(se[:], var[:], ACTF.Sqrt, bias=eps1[:])
            rrf = wrk.tile([1, S], F32, name=f"rf_{u}", tag="lnrf")
            nc.vector.reciprocal_approx_fast(rrf[:], se[:])
            rr = wrk.tile([1, 2 * S], F32R, name=f"rr_{u}", tag="lnrr")
            # the two f32r halves are produced in parallel branches
            nc.vector.tensor_copy(rr[0:1, 0:S], rrf[:])
            nc.vector.tensor_mul(rr[0:1, S:2 * S], rrf[:], s1[:])
            gc = pack[0:1, 144 + (l * 2 + i) * D:144 + (l * 2 + i + 1) * D]
            rb = ps_sm.tile([D, S], F32, name=f"rb_{u}", tag="ps_sm")
            rm = ps_sm.tile([D, S], F32, name=f"rm_{u}", tag="ps_sm")
            nc.tensor.matmul(rb[:], gc, rr[0:1, 0:S])      # g_d * r_s
            nc.tensor.matmul(rm[:], gc, rr[0:1, S:2 * S])  # g*r*mu
            t2 = wrk.tile([D, S], F32, name=f"t2_{u}", tag="lnt2")
            nc.vector.tensor_mul(t2[:], x[:], rb[:])
            bc = _f(pack[0:D, 165 + l * 2 + i:166 + l * 2 + i])
            if out4 is None:
                nc.vector.scalar_tensor_tensor(out_ap, t2[:], bc, rm[:],
                                               op0=ALU.add, op1=ALU.subtract)
            else:
                for q in range(QC):
                    nc.vector.scalar_tensor_tensor(
                        out4[32 * q:32 * q + D, :],
                        t2[:, q * 128:(q + 1) * 128], bc,
                        rm[:, q * 128:(q + 1) * 128],
                        op0=ALU.add, op1=ALU.subtract)

        def body(l, it, after_b=None):
            u = f"{l}_{it}"
            # ---- qk for both batches; v per batch-chunk ----
            qsb = wrk.tile([D, S2], F32R, name=f"qs{u}", tag="qsb")
            ksb = wrk.tile([D, S2], F32R, name=f"ks{u}", tag="ksb")
            for b in range(BPC):
                hb = h[:, b * S:(b + 1) * S]
                qkp = ps_sm.tile([37, S], F32, name=f"qk{u}{b}",
                                 tag="ps_sm")
                nc.tensor.matmul(qkp[:], wqk[l], hb)
                nc.scalar.copy(qsb[:, b * S:(b + 1) * S], qkp[0:D, :])
                nc.vector.tensor_copy(ksb[:, b * S:(b + 1) * S],
                                      qkp[32:32 + D, :])
            for b in range(BPC):
                hb = h[:, b * S:(b + 1) * S]
                vp = ps_sm.tile([128, QC, D], F32, name=f"vp{u}{b}",
                                tag="ps_sm")
                for kc in range(QC):
                    nc.tensor.matmul(vp[:, kc, :],
                                     _f(hb[:, kc * 128:(kc + 1) * 128]),
                                     wv[l])
                nc.vector.tensor_copy(vsb[b][:, :, 0:D], vp[:])

            # ---- attention (transposed, no-max softmax) ----
            rz = wrk.tile([1, S2], F32R, name=f"rz{u}", tag="rz")
            for b in range(BPC):
                bc0 = b * S
                cz = ps_sm.tile([33, S], F32, name=f"cz{u}{b}",
                                tag="ps_sm")
                for kc in range(QC):
                    # causal triangle: chunk kc's keys (k >= 128*kc) only
                    # reach queries q >= 128*kc -> compute/exp/mask/accum
                    # on the valid column range only (chunk 0 is full
                    # width with start=True, so has_written bits cover
                    # the whole bank; later chunks accumulate sub-ranges)
                    off = kc * 128
                    n = S - off
                    r32 = n >= 256  # fp32r ISA restriction for small N
                    scp = ps_big.tile([128, S], F32, name=f"sc{u}{b}{kc}",
                                     tag="big")
                    ka = ksb[:, bc0 + off:bc0 + off + 128]
                    qa = qsb[:, bc0 + off:bc0 + S]
                    if r32:
                        nc.tensor.matmul(scp[:, 0:n], ka, qa)
                    else:
                        nc.tensor.matmul(scp[:, 0:n], _f(ka), _f(qa))
                    eraw = att.tile([128, S], F32, name=f"er{u}{b}{kc}",
                                    tag="eraw")
                    nc.scalar.activation(eraw[:, 0:n], scp[:, 0:n],
                                         ACTF.Exp)
                    expT = att.tile([128, S], F32R, name=f"ex{u}{b}{kc}",
                                    tag="expT")
                    nc.vector.tensor_mul(expT[:, 0:n], eraw[:, 0:n],
                                         maskb[:, kc, off:S])
                    if r32:
                        nc.tensor.matmul(cz[:, off:S], vsb[b][:, kc, :],
                                         expT[:, 0:n],
                                         start=(kc == 0),
                                         stop=(kc == QC - 1))
                    else:
                        nc.tensor.matmul(cz[:, off:S],
                                         _f(vsb[b][:, kc, :]),
                                         _f(expT[:, 0:n]),
                                         start=(kc == 0),
                                         stop=(kc == QC - 1))
                zf = wrk.tile([1, S], F32, name=f"zf{u}{b}", tag="zf")
                nc.vector.tensor_copy(zf[:], cz[32:33, :])
                zr = wrk.tile([1, S], F32, name=f"zr{u}{b}", tag="zr")
                nc.vector.reciprocal_approx_fast(zr[:], zf[:])
                nc.vector.tensor_copy(rz[0:1, bc0:bc0 + S], zr[:])  # ->f32r
                rzb = ps_sm.tile([D, S], F32, name=f"rzb{u}{b}", tag="ps_sm")
                nc.tensor.matmul(rzb[:], ones5, rz[0:1, bc0:bc0 + S])
                ctxs = wrk.tile([D, S], F32, name=f"cs{u}{b}", tag="ctxs")
                nc.scalar.copy(ctxs[:], cz[0:D, :])
                nc.vector.tensor_mul(ctxa[0:D, bc0:bc0 + S],
                                     ctxs[:], rzb[:])

            # ---- out_proj + LN1 ----
            for b in range(BPC):
                bc0 = b * S
                pp = ps_sm.tile([D, S], F32, name=f"pp{u}{b}", tag="ps_sm")
                nc.tensor.matmul(pp[:], wo[l], ctxa[:, bc0:bc0 + S])
                layernorm(l, 0, b, it, h[0:D, bc0:bc0 + S], pp[:],
                          out_ap=h[0:D, bc0:bc0 + S])

            # ---- ffn + LN2 ----
            for b in range(BPC):
                bc0 = b * S
                f1p = ps_sm.tile([F, S], F32, name=f"f1{u}{b}", tag="ps_sm")
                nc.tensor.matmul(f1p[:], w1[l], h[:, bc0:bc0 + S])
                nc.scalar.activation(f1a[0:F, bc0:bc0 + S], f1p[:], ACTF.Relu)
            for b in range(BPC):
                bc0 = b * S
                f2p = ps_sm.tile([D, S], F32, name=f"f2{u}{b}", tag="ps_sm")
                nc.tensor.matmul(f2p[:], w2[l], f1a[:, bc0:bc0 + S])
                if l == L - 1:
                    layernorm(l, 1, b, it, h[0:D, bc0:bc0 + S], f2p[:],
                              out4=hfin4[b])
                else:
                    layernorm(l, 1, b, it, h[0:D, bc0:bc0 + S], f2p[:],
                              out_ap=h[0:D, bc0:bc0 + S])
                if after_b is not None:
                    after_b(b)

        def logits_b(b, it):
            # batch-major: stores for batch b start right after its LN2.
            # per 2-vocab-chunk group: 8 row-tiled matmuls (4 seq chunks
            # run concurrently in the PE array via tile_position, each into
            # its own single-bank psum tile) -> fp16 stage -> one 1MB
            # contiguous store, alternating HWDGE rings
            for g in range(NVC // 2):
                st = stg.tile([128, 2, QC, VCH], F16,
                              name=f"st{b}{g}_{it}", tag="stage")
                for vv in range(2):
                    vc = 2 * g + vv
                    lps = []
                    for i in range(QC):
                        lp = ps_big.tile([128, VCH], F32,
                                         name=f"lp{vc}{b}{i}_{it}",
                                         tag="big")
                        nc.tensor.matmul(
                            lp[:],
                            hfin4[b][32 * i:32 * i + D + 1, :],
                            fcw4[32 * i:32 * i + D + 1,
                                 vc * VCH:(vc + 1) * VCH],
                            tile_position=(32 * i, 0))
                        lps.append(lp)
                    # split copies 2:2 DVE:ACT (both stay under DMA)
                    for i in range(QC):
                        if i >= 2:
                            nc.scalar.copy(st[:, vv, i, :], lps[i][:])
                        else:
                            nc.vector.tensor_copy(st[:, vv, i, :],
                                                  lps[i][:])
                eng = nc.sync if g % 2 == 0 else nc.scalar
                eng.dma_start(d_out[:, b, g, :], st[:])

        for it in range(iters):
            if it > 0:
                nc.sync.dma_start(h[:], d_h0[:])
            for l in range(L):
                if l == L - 1:
                    body(l, it, after_b=lambda b, it=it: logits_b(b, it))
                else:
                    body(l, it)

    nc.compile()
    return nc


def _get_program(iters=1):
    if iters not in _CACHED:
        _CACHED[iters] = _build_program(iters)
    return _CACHED[iters]


def _pos_encoding_np():
    pos = np.arange(B, dtype=np.float32)[:, None]
    div = np.exp(np.arange(0, D, 2, dtype=np.float32)
                 * (-math.log(10000.0) / D))
    pe = np.zeros((B, D), dtype=np.float32)
    pe[:, 0::2] = np.sin(pos * div)
    pe[:, 1::2] = np.cos(pos * div[:-1])
    return pe


def host_inputs(x, emb, in_proj_w, in_proj_b, out_proj_w, out_proj_b,
                ln1_g, ln1_b, ln2_g, ln2_b, ff1_w, ff1_b, ff2_w, ff2_b,
                fc_w, fc_b):
    """Build the per-core input maps (host-side prep only)."""
    x = np.asarray(x).astype(np.int64)
    emb = np.asarray(emb, dtype=np.float32)
    f32 = lambda a: np.ascontiguousarray(np.asarray(a, dtype=np.float32))
    in_proj_w, in_proj_b = f32(in_proj_w), f32(in_proj_b)
    out_proj_w, out_proj_b = f32(out_proj_w), f32(out_proj_b)
    ff1_w, ff1_b, ff2_w, ff2_b = f32(ff1_w), f32(ff1_b), f32(ff2_w), f32(ff2_b)
    ln1_g, ln1_b, ln2_g, ln2_b = f32(ln1_g), f32(ln1_b), f32(ln2_g), f32(ln2_b)
    fc_w, fc_b = f32(fc_w), f32(fc_b)

    h0 = emb[x] * np.float32(SQRT_D)          # [B, S, D]
    h0 = h0 + _pos_encoding_np()[:, None, :]  # pe-by-batch-index (faithful)
    h0t = np.transpose(h0, (0, 2, 1))         # [B, D, S]

    def aug(wT, bias):  # [K, M] + bias row
        return np.ascontiguousarray(
            np.concatenate([wT, bias[None, :]], axis=0).astype(np.float32))

    # single packed small-constant buffer [F+1, 174]:
    #  cols 0-73: wqk per layer [6,37] (q pre-scaled by 1/sqrt(D) at 0-4,
    #             k at 32-36 so the psum read of k is 32-aligned)
    #  74-83: wv, 84-93: wo, 94-133: w1, 134-143: w2 (rows 0-20)
    #  144-163: ln gains, 164: c02, 165-168: ln biases, 169-173: ones
    packw = np.zeros((F + 1, 174), np.float32)
    for l in range(L):
        packw[0:D + 1, 37 * l:37 * l + D] = aug(
            in_proj_w[l][0:D].T * SCALE, in_proj_b[l][0:D] * SCALE)
        packw[0:D + 1, 37 * l + 32:37 * (l + 1)] = aug(
            in_proj_w[l][D:2 * D].T, in_proj_b[l][D:2 * D])
        packw[0:D + 1, 74 + D * l:74 + D * (l + 1)] = aug(
            in_proj_w[l][2 * D:3 * D].T, in_proj_b[l][2 * D:3 * D])
        packw[0:D + 1, 84 + D * l:84 + D * (l + 1)] = aug(
            out_proj_w[l].T, out_proj_b[l])
        packw[0:D + 1, 94 + F * l:94 + F * (l + 1)] = aug(
            ff1_w[l].T, ff1_b[l])
        packw[0:F + 1, 134 + D * l:134 + D * (l + 1)] = aug(
            ff2_w[l].T, ff2_b[l])
        packw[0, 144 + (2 * l) * D:144 + (2 * l + 1) * D] = ln1_g[l]
        packw[0, 144 + (2 * l + 1) * D:144 + (2 * l + 2) * D] = ln2_g[l]
        packw[0:D, 165 + 2 * l] = ln1_b[l]
        packw[0:D, 165 + 2 * l + 1] = ln2_b[l]
    packw[0:D, 164] = 1.0 / D
    packw[0, 169:174] = 1.0

    # binary causal mask, [k, q] layout per 128-row k-chunk
    kidx = np.arange(S)
    maskf = np.zeros((128, QC, S), np.float32)
    for kc in range(QC):
        kpos = kc * 128 + np.arange(128)[:, None]
        maskf[:, kc, :] = (kidx[None, :] >= kpos).astype(np.float32)
    fcw = np.ascontiguousarray(
        np.concatenate([fc_w.T, fc_b[None, :]], axis=0).astype(np.float32))

    # one-time SBUF init patterns (loaded by DMA, not engine-written)
    ctxa0 = np.ones((D + 1, S2), np.float32)
    f1a0 = np.ones((F + 1, S2), np.float32)
    vsb0 = np.zeros((128, QC, 33), np.float32)
    vsb0[:, :, 32] = 1.0
    hf0 = np.ones((102, 128), np.float32)

    shared = dict(packw=packw, mask=maskf, fcw=fcw,
                  ctxa0=ctxa0, f1a0=f1a0, vsb0=vsb0, hf0=hf0)
    in_maps = []
    for c in range(NCORES):
        hh = np.ones((D + 1, S2), np.float32)
        for b in range(BPC):
            hh[0:D, b * S:(b + 1) * S] = h0t[c * BPC + b]
        in_maps.append(dict(h0=hh, **shared))
    return in_maps


def run(in_maps, trace=False, iters=1, **kw):
    nc = _get_program(iters)
    return run_bass_kernel_spmd(nc, in_maps, list(range(NCORES)),
                                trace=trace, **kw)


def unshard(res):
    """Per-core [128, BPC, 8, 2, QC, VCH] fp16 -> [B, S, V] fp32."""
    outs = []
    for c in range(NCORES):
        a = np.asarray(res.results[c]["out"]).astype(np.float32)
        a = a.reshape(128, BPC, NVC // 2, 2, QC, VCH)
        a = np.transpose(a, (1, 4, 0, 2, 3, 5)).reshape(BPC, S, V)
        outs.append(a)
    return np.ascontiguousarray(np.concatenate(outs, axis=0))


def kernel(**inputs) -> np.ndarray:
    in_maps = host_inputs(**inputs)
    res = run(in_maps)
    return unshard(res)


if __name__ == "__main__":
    import reference
    ins = {k: np.asarray(v) for k, v in reference.setup_inputs().items()}
    got = kernel(**ins)
    exp = np.asarray(reference.reference(**reference.setup_inputs()))
    err = np.abs(got - exp)
    rel = err.max() / (np.abs(exp).max() + 1e-30)
    print("max abs err:", err.max(), "rel:", rel)



# revision 1
# speedup vs baseline: 1.0972x; 1.0972x over previous
"""MicroTransformer (B=16,S=512,V=8000,D=5,F=20,L=2) on 8 trn2 NeuronCores.

Sharding: pure data parallel over batch (2 batch elements per core).
All parameters replicated. Whole transformer body + logits matmul run on
device; host only does input prep (embedding row gather, positional
encoding constant, weight layout transforms) and the final reshape.

Per-core device program (Bass/Tile, fully unrolled):
  state h [6, 1024] f32r: rows 0-4 = h^T for batch0|batch1, row 5 = ones
  (bias row for augmented matmuls).  All small constants arrive in one
  packed [21,174] DMA load; aug-tile ones rows / V zero-padding are
  DMA-loaded init patterns (single-row engine writes are illegal:
  engine APs must start at 32-aligned partitions).
  Attention is computed fully transposed ([k, q] layout), softmax without
  row-max (scores are bounded, |s| < 64 by construction of the inputs):
    qk       = Wqk_aug [6,37] x h-half  (q cols 0-4 pre-scaled by
               1/sqrt(D), k at cols 32-36 so its psum read is 32-aligned)
    scoresT  = k-chunk [5,128] x q [5,512]       -> PSUM [128k, 512q]
    eraw     = ACT Exp(scoresT)                  (PSUM -> SBUF f32)
    expT     = eraw * binary-causal-mask         (DVE, rounds to f32r)
    ctxZ     = sum_kc V_aug-chunk [128,33] x expT -> PSUM [33,512]
               (V has a ones column at 32 => partition 32 accumulates Z)
    1/Z      = reciprocal_approx_fast; bcast to [5,512] via a K=1 matmul
    proj/ffn = augmented f32r matmuls; LayerNorm via matmul stats
               (rsqrt = ACT Sqrt + DVE reciprocal_approx_fast)
  logits: final h is scattered to hfin4 [102,128] (4 seq-chunks at
  partition offsets 0/32/64/96); fcw4 holds fc_w_aug replicated at the
  same offsets.  Per 500-wide vocab chunk, 4 row-tiled matmuls
  (tile_position=(32i,0)) run concurrently in the PE array, each into
  its own single-bank psum tile (several row-tiled matmuls into one
  multi-bank tile crash at runtime).  Copies (2 DVE / 2 ACT) cast to
  fp16 stage tiles; per batch and 2-vocab-chunk group one contiguous
  1MB store (alternating HWDGE rings) writes out
  [128, 2, 8, 4000] fp16, so batch-0 stores overlap batch-1 tail work.
"""

import math

import numpy as np

import concourse.bacc as bacc
import concourse.bass as bass
import concourse.mybir as mybir
import concourse.tile as tile
from concourse.bass_utils import run_bass_kernel_spmd

F32 = mybir.dt.float32
F32R = mybir.dt.float32r
BF16 = mybir.dt.bfloat16
F16 = mybir.dt.float16
ALU = mybir.AluOpType
ACTF = mybir.ActivationFunctionType

def _r(ap):
    """float32r view: 4x PE throughput vs fp32 at moving size >= 256."""
    return ap.bitcast(F32R)


def _f(ap):
    """plain-f32 view (for tiny matmuls where fp32r is ISA-restricted)."""
    return ap.bitcast(F32)


B, S, V, D, F, L = 16, 512, 8000, 5, 20, 2
EPS = 1e-5
NCORES = 8
BPC = B // NCORES  # batches per core = 2
SQRT_D = math.sqrt(float(D))
SCALE = 1.0 / SQRT_D
QC = S // 128                  # 4 seq chunks of 128
VCH = 500                      # vocab chunk per matmul (<=512 psum bank)
NVC = V // VCH                 # 16
S2 = BPC * S                   # 1024

_CACHED = {}  # iters -> nc


def _build_program(iters=1):
    nc = bacc.Bacc("TRN2", target_bir_lowering=False, debug=False,
                   num_devices=NCORES)

    # ---- DRAM I/O ----
    d_h0 = nc.dram_tensor("h0", [D + 1, S2], F32R, kind="ExternalInput")
    d_pack = nc.dram_tensor("packw", [F + 1, 174], F32R,
                            kind="ExternalInput")
    d_mask = nc.dram_tensor("mask", [128, QC, S], F32, kind="ExternalInput")
    d_ctxa0 = nc.dram_tensor("ctxa0", [D + 1, S2], F32R,
                             kind="ExternalInput")
    d_f1a0 = nc.dram_tensor("f1a0", [F + 1, S2], F32R, kind="ExternalInput")
    d_vsb0 = nc.dram_tensor("vsb0", [128, QC, 33], F32R,
                            kind="ExternalInput")
    d_hf0 = nc.dram_tensor("hf0", [102, 128], F32R, kind="ExternalInput")
    d_fcw = nc.dram_tensor("fcw", [D + 1, V], F32R, kind="ExternalInput")
    d_out = nc.dram_tensor("out", [128, BPC, NVC // 2, 2 * QC * VCH],
                           F16, kind="ExternalOutput")

    from contextlib import ExitStack
    with tile.TileContext(nc) as tc, ExitStack() as es, \
            nc.allow_low_precision(reason="f32r/bf16/f16 rounding intended"):
        cst = es.enter_context(tc.tile_pool(name="cst", bufs=1))
        wrk = es.enter_context(tc.tile_pool(name="wrk", bufs=2))
        att = es.enter_context(tc.tile_pool(name="att", bufs=3))
        stg = es.enter_context(tc.tile_pool(name="stg", bufs=4))
        ps_big = es.enter_context(tc.tile_pool(name="ps_big", bufs=5,
                                               space="PSUM"))
        ps_sm = es.enter_context(tc.tile_pool(name="ps_sm", bufs=3,
                                              space="PSUM"))

        # ---- constants into SBUF (once) ----
        # ordering: h + one packed weight load go first on the sync ring so
        # the body can start immediately; the big mask/fcw4 loads follow.
        h = cst.tile([D + 1, S2], F32R, name="h", tag="h")
        nc.sync.dma_start(h[:], d_h0[:])
        pack = cst.tile([F + 1, 174], F32R, name="pack", tag="pack")
        nc.sync.dma_start(pack[:], d_pack[:])
        wqk = [pack[0:D + 1, 37 * l:37 * (l + 1)] for l in range(L)]
        wv = [_f(pack[0:D + 1, 74 + D * l:74 + D * (l + 1)])
              for l in range(L)]
        wo = [pack[0:D + 1, 84 + D * l:84 + D * (l + 1)] for l in range(L)]
        w1 = [pack[0:D + 1, 94 + F * l:94 + F * (l + 1)] for l in range(L)]
        w2 = [pack[0:F + 1, 134 + D * l:134 + D * (l + 1)] for l in range(L)]
        c02 = pack[0:D, 164:165]
        ones5 = pack[0:1, 169:169 + D]
        eps1 = cst.tile([1, 1], F32, name="eps1", tag="eps1")
        nc.vector.memset(eps1[:], EPS)
        maskb = cst.tile([128, QC, S], F32, name="maskb", tag="maskb")
        nc.scalar.dma_start(maskb[:], d_mask[:])
        fcw4 = cst.tile([102, V], F32R, name="fcw4", tag="fcw4")
        for i in range(QC):
            nc.sync.dma_start(fcw4[32 * i:32 * i + D + 1, :], d_fcw[:])
        # dedicated aug tiles (f32r): ones rows / zero padding loaded from
        # DRAM once (engine writes to single rows would be partition-
        # unaligned and memset can't write f32r); data rows are fully
        # rewritten every use, so the init patterns persist.
        ctxa = cst.tile([D + 1, S2], F32R, name="ctxa", tag="ctxa")
        nc.scalar.dma_start(ctxa[:], d_ctxa0[:])
        f1a = cst.tile([F + 1, S2], F32R, name="f1a", tag="f1a")
        nc.scalar.dma_start(f1a[:], d_f1a0[:])
        vsb = []
        for b in range(BPC):
            # V augmented: cols 0-4 = V, cols 5-31 = 0, col 32 = ones
            # => ctxZ matmul puts Z at out partition 32 (32-aligned read)
            t = cst.tile([128, QC, 33], F32R, name=f"vsb{b}", tag=f"vsb{b}")
            nc.scalar.dma_start(t[:], d_vsb0[:])
            vsb.append(t)
        hfin4 = []
        for b in range(BPC):
            t = cst.tile([102, 128], F32R, name=f"hfin4{b}", tag=f"hfin4{b}")
            nc.scalar.dma_start(t[:], d_hf0[:])
            hfin4.append(t)

        def layernorm(l, i, b, it, resid_ap, add_ps, out4=None, out_ap=None):
            """out = LN(resid + add) * g + b.

            out4: write result into hfin4[b] (4 chunk slices); else out_ap.
            """
            u = f"{l}{i}{b}_{it}"
            # chain kept on DVE where possible (same-engine deps avoid
            # cross-engine semaphore hops); only sqrt needs ACT
            x = wrk.tile([D, S], F32R, name=f"lnx{u}", tag="lnx")
            nc.vector.tensor_add(x[:], resid_ap, add_ps)
            xsq = wrk.tile([D, S], F32R, name=f"lnq{u}", tag="lnq")
            nc.vector.tensor_mul(xsq[:], x[:], x[:])
            s1 = ps_sm.tile([1, S], F32, name=f"s1_{u}", tag="ps_sm")
            s2 = ps_sm.tile([1, S], F32, name=f"s2_{u}", tag="ps_sm")
            nc.tensor.matmul(s1[:], c02, x[:])      # mean
            nc.tensor.matmul(s2[:], c02, xsq[:])    # E[x^2]
            t1 = wrk.tile([1, S], F32, name=f"t1_{u}", tag="lnt1")
            nc.scalar.square(t1[:], s1[:])             # mean^2 (ACT;
            # tensor_mul(s1, s1) would read PSUM twice in one op)
            var = wrk.tile([1, S], F32, name=f"lv_{u}", tag="lnvar")
            nc.vector.tensor_sub(var[:], s2[:], t1[:])
            se = wrk.tile([1, S], F32, name=f"se_{u}", tag="lnse")
            nc.scalar.activation(se[:], var[:], ACTF.Sqrt, bias=eps1[:])
            rrf = wrk.tile([1, S], F32, name=f"rf_{u}", tag="lnrf")
            nc.vector.reciprocal_approx_fast(rrf[:], se[:])
            rr = wrk.tile([1, 2 * S], F32R, name=f"rr_{u}", tag="lnrr")
            # the two f32r halves are produced in parallel branches
            nc.vector.tensor_copy(rr[0:1, 0:S], rrf[:])
            nc.vector.tensor_mul(rr[0:1, S:2 * S], rrf[:], s1[:])
            gc = pack[0:1, 144 + (l * 2 + i) * D:144 + (l * 2 + i + 1) * D]
            rb = ps_sm.tile([D, S], F32, name=f"rb_{u}", tag="ps_sm")
            rm = ps_sm.tile([D, S], F32, name=f"rm_{u}", tag="ps_sm")
            nc.tensor.matmul(rb[:], gc, rr[0:1, 0:S])      # g_d * r_s
            nc.tensor.matmul(rm[:], gc, rr[0:1, S:2 * S])  # g*r*mu
            t2 = wrk.tile([D, S], F32, name=f"t2_{u}", tag="lnt2")
            nc.vector.tensor_mul(t2[:], x[:], rb[:])
            bc = _f(pack[0:D, 165 + l * 2 + i:166 + l * 2 + i])
            if out4 is None:
                nc.vector.scalar_tensor_tensor(out_ap, t2[:], bc, rm[:],
                                               op0=ALU.add, op1=ALU.subtract)
            else:
                for q in range(QC):
                    nc.vector.scalar_tensor_tensor(
                        out4[32 * q:32 * q + D, :],
                        t2[:, q * 128:(q + 1) * 128], bc,
                        rm[:, q * 128:(q + 1) * 128],
                        op0=ALU.add, op1=ALU.subtract)

        def body(l, it, after_b=None):
            u = f"{l}_{it}"
            # ---- qk for both batches; v per batch-chunk ----
            qsb = wrk.tile([D, S2], F32R, name=f"qs{u}", tag="qsb")
            ksb = wrk.tile([D, S2], F32R, name=f"ks{u}", tag="ksb")
            for b in range(BPC):
                hb = h[:, b * S:(b + 1) * S]
                qkp = ps_sm.tile([37, S], F32, name=f"qk{u}{b}",
                                 tag="ps_sm")
                nc.tensor.matmul(qkp[:], wqk[l], hb)
                nc.scalar.copy(qsb[:, b * S:(b + 1) * S], qkp[0:D, :])
                nc.vector.tensor_copy(ksb[:, b * S:(b + 1) * S],
                                      qkp[32:32 + D, :])
            for b in range(BPC):
                hb = h[:, b * S:(b + 1) * S]
                vp = ps_sm.tile([128, QC, D], F32, name=f"vp{u}{b}",
                                tag="ps_sm")
                for kc in range(QC):
                    nc.tensor.matmul(vp[:, kc, :],
                                     _f(hb[:, kc * 128:(kc + 1) * 128]),
                                     wv[l])
                nc.vector.tensor_copy(vsb[b][:, :, 0:D], vp[:])

            # ---- attention (transposed, no-max softmax) ----
            rz = wrk.tile([1, S2], F32R, name=f"rz{u}", tag="rz")
            for b in range(BPC):
                bc0 = b * S
                cz = ps_sm.tile([33, S], F32, name=f"cz{u}{b}",
                                tag="ps_sm")
                for kc in range(QC):
                    # causal triangle: chunk kc's keys (k >= 128*kc) only
                    # reach queries q >= 128*kc -> compute/exp/mask/accum
                    # on the valid column range only (chunk 0 is full
                    # width with start=True, so has_written bits cover
                    # the whole bank; later chunks accumulate sub-ranges)
                    off = kc * 128
                    n = S - off
                    r32 = n >= 256  # fp32r ISA restriction for small N
                    scp = ps_big.tile([128, S], F32, name=f"sc{u}{b}{kc}",
                                     tag="big")
                    ka = ksb[:, bc0 + off:bc0 + off + 128]
                    qa = qsb[:, bc0 + off:bc0 + S]
                    if r32:
                        nc.tensor.matmul(scp[:, 0:n], ka, qa)
                    else:
                        nc.tensor.matmul(scp[:, 0:n], _f(ka), _f(qa))
                    eraw = att.tile([128, S], F32, name=f"er{u}{b}{kc}",
                                    tag="eraw")
                    nc.scalar.activation(eraw[:, 0:n], scp[:, 0:n],
                                         ACTF.Exp)
                    expT = att.tile([128, S], F32R, name=f"ex{u}{b}{kc}",
                                    tag="expT")
                    nc.vector.tensor_mul(expT[:, 0:n], eraw[:, 0:n],
                                         maskb[:, kc, off:S])
                    if r32:
                        nc.tensor.matmul(cz[:, off:S], vsb[b][:, kc, :],
                                         expT[:, 0:n],
                                         start=(kc == 0),
                                         stop=(kc == QC - 1))
                    else:
                        nc.tensor.matmul(cz[:, off:S],
                                         _f(vsb[b][:, kc, :]),
                                         _f(expT[:, 0:n]),
                                         start=(kc == 0),
                                         stop=(kc == QC - 1))
                zf = wrk.tile([1, S], F32, name=f"zf{u}{b}", tag="zf")
                nc.vector.tensor_copy(zf[:], cz[32:33, :])
                zr = wrk.tile([1, S], F32, name=f"zr{u}{b}", tag="zr")
                nc.vector.reciprocal_approx_fast(zr[:], zf[:])
                nc.vector.tensor_copy(rz[0:1, bc0:bc0 + S], zr[:])  # ->f32r
                rzb = ps_sm.tile([D, S], F32, name=f"rzb{u}{b}", tag="ps_sm")
                nc.tensor.matmul(rzb[:], ones5, rz[0:1, bc0:bc0 + S])
                ctxs = wrk.tile([D, S], F32, name=f"cs{u}{b}", tag="ctxs")
                nc.scalar.copy(ctxs[:], cz[0:D, :])
                nc.vector.tensor_mul(ctxa[0:D, bc0:bc0 + S],
                                     ctxs[:], rzb[:])

            # ---- out_proj + LN1 ----
            for b in range(BPC):
                bc0 = b * S
                pp = ps_sm.tile([D, S], F32, name=f"pp{u}{b}", tag="ps_sm")
                nc.tensor.matmul(pp[:], wo[l], ctxa[:, bc0:bc0 + S])
                layernorm(l, 0, b, it, h[0:D, bc0:bc0 + S], pp[:],
                          out_ap=h[0:D, bc0:bc0 + S])

            # ---- ffn + LN2 ----
            for b in range(BPC):
                bc0 = b * S
                f1p = ps_sm.tile([F, S], F32, name=f"f1{u}{b}", tag="ps_sm")
                nc.tensor.matmul(f1p[:], w1[l], h[:, bc0:bc0 + S])
                nc.scalar.activation(f1a[0:F, bc0:bc0 + S], f1p[:], ACTF.Relu)
            for b in range(BPC):
                bc0 = b * S
                f2p = ps_sm.tile([D, S], F32, name=f"f2{u}{b}", tag="ps_sm")
                nc.tensor.matmul(f2p[:], w2[l], f1a[:, bc0:bc0 + S])
                if l == L - 1:
                    layernorm(l, 1, b, it, h[0:D, bc0:bc0 + S], f2p[:],
                              out4=hfin4[b])
                else:
                    layernorm(l, 1, b, it, h[0:D, bc0:bc0 + S], f2p[:],
                              out_ap=h[0:D, bc0:bc0 + S])
                if after_b is not None:
                    after_b(b)

        def logits_b(b, it):
            # batch-major: stores for batch b start right after its LN2.
            # per 2-vocab-chunk group: 8 row-tiled matmuls (4 seq chunks
            # run concurrently in the PE array via tile_position, each into
            # its own single-bank psum tile) -> fp16 stage -> one 1MB
            # contiguous store, alternating HWDGE rings
            for g in range(NVC // 2):
                st = stg.tile([128, 2, QC, VCH], F16,
                              name=f"st{b}{g}_{it}", tag="stage")
                for vv in range(2):
                    vc = 2 * g + vv
                    lps = []
                    for i in range(QC):
                        lp = ps_big.tile([128, VCH], F32,
                                         name=f"lp{vc}{b}{i}_{it}",
                                         tag="big")
                        nc.tensor.matmul(
                            lp[:],
                            hfin4[b][32 * i:32 * i + D + 1, :],
                            fcw4[32 * i:32 * i + D + 1,
                                 vc * VCH:(vc + 1) * VCH],
                            tile_position=(32 * i, 0))
                        lps.append(lp)
                    # split copies 2:2 DVE:ACT (both stay under DMA)
                    for i in range(QC):
                        if i >= 2:
                            nc.scalar.copy(st[:, vv, i, :], lps[i][:])
                        else:
                            nc.vector.tensor_copy(st[:, vv, i, :],
                                                  lps[i][:])
                eng = nc.sync if g % 2 == 0 else nc.scalar
                eng.dma_start(d_out[:, b, g, :], st[:])

        for it in range(iters):
            if it > 0:
                nc.sync.dma_start(h[:], d_h0[:])
            for l in range(L):
                if l == L - 1:
                    body(l, it, after_b=lambda b, it=it: logits_b(b, it))
                else:
                    body(l, it)

    nc.compile()
    return nc


def _get_program(iters=1):
    if iters not in _CACHED:
        _CACHED[iters] = _build_program(iters)
    return _CACHED[iters]


def _pos_encoding_np():
    pos = np.arange(B, dtype=np.float32)[:, None]
    div = np.exp(np.arange(0, D, 2, dtype=np.float32)
                 * (-math.log(10000.0) / D))
    pe = np.zeros((B, D), dtype=np.float32)
    pe[:, 0::2] = np.sin(pos * div)
    pe[:, 1::2] = np.cos(pos * div[:-1])
    return pe


def host_inputs(x, emb, in_proj_w, in_proj_b, out_proj_w, out_proj_b,
                ln1_g, ln1_b, ln2_g, ln2_b, ff1_w, ff1_b, ff2_w, ff2_b,
                fc_w, fc_b):
    """Build the per-core input maps (host-side prep only)."""
    x = np.asarray(x).astype(np.int64)
    emb = np.asarray(emb, dtype=np.float32)
    f32 = lambda a: np.ascontiguousarray(np.asarray(a, dtype=np.float32))
    in_proj_w, in_proj_b = f32(in_proj_w), f32(in_proj_b)
    out_proj_w, out_proj_b = f32(out_proj_w), f32(out_proj_b)
    ff1_w, ff1_b, ff2_w, ff2_b = f32(ff1_w), f32(ff1_b), f32(ff2_w), f32(ff2_b)
    ln1_g, ln1_b, ln2_g, ln2_b = f32(ln1_g), f32(ln1_b), f32(ln2_g), f32(ln2_b)
    fc_w, fc_b = f32(fc_w), f32(fc_b)

    h0 = emb[x] * np.float32(SQRT_D)          # [B, S, D]
    h0 = h0 + _pos_encoding_np()[:, None, :]  # pe-by-batch-index (faithful)
    h0t = np.transpose(h0, (0, 2, 1))         # [B, D, S]

    def aug(wT, bias):  # [K, M] + bias row
        return np.ascontiguousarray(
            np.concatenate([wT, bias[None, :]], axis=0).astype(np.float32))

    # single packed small-constant buffer [F+1, 174]:
    #  cols 0-73: wqk per layer [6,37] (q pre-scaled by 1/sqrt(D) at 0-4,
    #             k at 32-36 so the psum read of k is 32-aligned)
    #  74-83: wv, 84-93: wo, 94-133: w1, 134-143: w2 (rows 0-20)
    #  144-163: ln gains, 164: c02, 165-168: ln biases, 169-173: ones
    packw = np.zeros((F + 1, 174), np.float32)
    for l in range(L):
        packw[0:D + 1, 37 * l:37 * l + D] = aug(
            in_proj_w[l][0:D].T * SCALE, in_proj_b[l][0:D] * SCALE)
        packw[0:D + 1, 37 * l + 32:37 * (l + 1)] = aug(
            in_proj_w[l][D:2 * D].T, in_proj_b[l][D:2 * D])
        packw[0:D + 1, 74 + D * l:74 + D * (l + 1)] = aug(
            in_proj_w[l][2 * D:3 * D].T, in_proj_b[l][2 * D:3 * D])
        packw[0:D + 1, 84 + D * l:84 + D * (l + 1)] = aug(
            out_proj_w[l].T, out_proj_b[l])
        packw[0:D + 1, 94 + F * l:94 + F * (l + 1)] = aug(
            ff1_w[l].T, ff1_b[l])
        packw[0:F + 1, 134 + D * l:134 + D * (l + 1)] = aug(
            ff2_w[l].T, ff2_b[l])
        packw[0, 144 + (2 * l) * D:144 + (2 * l + 1) * D] = ln1_g[l]
        packw[0, 144 + (2 * l + 1) * D:144 + (2 * l + 2) * D] = ln2_g[l]
        packw[0:D, 165 + 2 * l] = ln1_b[l]
        packw[0:D, 165 + 2 * l + 1] = ln2_b[l]
    packw[0:D, 164] = 1.0 / D
    packw[0, 169:174] = 1.0

    # binary causal mask, [k, q] layout per 128-row k-chunk
    kidx = np.arange(S)
    maskf = np.zeros((128, QC, S), np.float32)
    for kc in range(QC):
        kpos = kc * 128 + np.arange(128)[:, None]
        maskf[:, kc, :] = (kidx[None, :] >= kpos).astype(np.float32)
    fcw = np.ascontiguousarray(
        np.concatenate([fc_w.T, fc_b[None, :]], axis=0).astype(np.float32))

    # one-time SBUF init patterns (loaded by DMA, not engine-written)
    ctxa0 = np.ones((D + 1, S2), np.float32)
    f1a0 = np.ones((F + 1, S2), np.float32)
    vsb0 = np.zeros((128, QC, 33), np.float32)
    vsb0[:, :, 32] = 1.0
    hf0 = np.ones((102, 128), np.float32)

    shared = dict(packw=packw, mask=maskf, fcw=fcw,
                  ctxa0=ctxa0, f1a0=f1a0, vsb0=vsb0, hf0=hf0)
    in_maps = []
    for c in range(NCORES):
        hh = np.ones((D + 1, S2), np.float32)
        for b in range(BPC):
            hh[0:D, b * S:(b + 1) * S] = h0t[c * BPC + b]
        in_maps.append(dict(h0=hh, **shared))
    return in_maps


def run(in_maps, trace=False, iters=1, **kw):
    nc = _get_program(iters)
    return run_bass_kernel_spmd(nc, in_maps, list(range(NCORES)),
                                trace=trace, **kw)


def unshard(res):
    """Per-core [128, BPC, 8, 2, QC, VCH] fp16 -> [B, S, V] fp32."""
    outs = []
    for c in range(NCORES):
        a = np.asarray(res.results[c]["out"]).astype(np.float32)
        a = a.reshape(128, BPC, NVC // 2, 2, QC, VCH)
        a = np.transpose(a, (1, 4, 0, 2, 3, 5)).reshape(BPC, S, V)
        outs.append(a)
    return np.ascontiguousarray(np.concatenate(outs, axis=0))


def kernel(**inputs) -> np.ndarray:
    in_maps = host_inputs(**inputs)
    res = run(in_maps)
    return unshard(res)


if __name__ == "__main__":
    import reference
    ins = {k: np.asarray(v) for k, v in reference.setup_inputs().items()}
    got = kernel(**ins)
    exp = np.asarray(reference.reference(**reference.setup_inputs()))
    err = np.abs(got - exp)
    rel = err.max() / (np.abs(exp).max() + 1e-30)
    print("max abs err:", err.max(), "rel:", rel)

